# revision 16
# baseline (speedup 1.0000x reference)
"""Trainium2 Bass kernel for nn_BatchMatMulModule.

Computes out = einsum("bnij,bmj->bnmi", x, y) with
  x: [4, 64, 3, 3] f32, y: [4, 100000, 3] f32 -> out: [4, 64, 100000, 3] f32.

The output (307 MB) dwarfs the inputs (4.8 MB); per-core store floor is
~107 us (38.4 MB at ~358 GB/s HBM per NeuronCore). The v1 kernel was
DVE-bound at ~165 us because both accumulate passes were fp32
scalar_tensor_tensor ops (1x mode, ~1.04 ns/elem each). This version
restructures the compute around DVE perf modes:

- y is shipped from the host as bf16 *planes* (yt = y[b].T, [3, 100000])
  so every compute read is contiguous; x scalars stay fp32 (scalar
  operands are exempt from the 2x-mode dtype rule).
- Per output element: two bf16 products (DVE tensor_scalar runs 4x: 0.26
  ns/elem; or ACT activation: 1x @1.2 GHz, 0.83 ns/elem), one bf16
  tensor_tensor add (DVE 2x: 0.52 ns/elem), and one final
  scalar_tensor_tensor that fuses the third product with the accumulate
  and the strided fp32 interleave write (1x, 1.04 ns/elem - the
  interleave into the [.., m, 3] output layout forces 1x regardless, so
  only ONE such pass remains vs v1's two).
- Chains are assigned per-engine via CHAIN_CODES to balance ACT vs DVE
  busy time. GPSIMD routing (add='P', fin='P'/'Q') was measured on HW and
  REGRESSES badly (127us -> 180+us for 12 GPS adds): the Q7 cores share
  their SBUF port with the DVE, so Pool-engine tensor ops stall the
  saturated DVE. Keep GPSIMD out of the hot loop.

The balanced ACT/DVE floor of this decomposition is ~126 us busy per
engine (store floor ~107 us at ~358 GB/s); measured ~127 us steady-state
per workload on quiet hardware.

Sharding: core c handles b = c // 2, n in [32 * (c % 2), ...) as in v1.
Per core, partition p = (a in 0..8, s in 0..16); group g covers n =
g*8 + a; segment s covers y rows [s*6250, (s+1)*6250); each unit
(g, h in 0..2) computes rows bounds[h]..bounds[h+1] (3124/3126 split so
bf16 packed modes keep even element counts and 4B alignment).
"""

import numpy as np
import ml_dtypes

import concourse.bacc as bacc
import concourse.mybir as mybir
from concourse.bass_utils import run_bass_kernel_spmd
from concourse.tile import TileContext

N_CORES = 8
P = 128
N_PER_CORE = 32
N_SUB = 8            # n values per group (partition-major)
SEGS = 16            # m segments per partition group
N_GROUPS = N_PER_CORE // N_SUB   # 4
M = 100000
ROWS = M // SEGS     # 6250 rows per segment (even)
BOUNDS = (0, 3124, ROWS)  # unit halves; both chunks even-sized
N_HALVES = len(BOUNDS) - 1

TRACE = False
LAST = None

_CACHED_NC = None

# Per-chain engine assignment. Chain index = (g * N_HALVES + h) * 3 + i.
# Code = 5 chars (p0, p1, add, fin, p2):
#   p0/p1: engine for the j=0 / j=1 products: 'A' (ACT) or 'V' (DVE ts)
#   add:   engine for the j0+j1 add: 'V' (DVE tt) or 'P' (GPSIMD tt)
#   fin:   final op producing ov[:, :, i] (fp32, stride-3 interleave):
#          'V' = DVE stt (fuses j2 product into the 1x strided write)
#          'P' = GPSIMD tt a+b2 (needs explicit j2 product, see p2)
#          'A' = DVE tt a+b2 -> v (bf16 2x), then ACT strided upcast copy
#          'Q' = like 'A' but the strided copy runs on GPSIMD
#   p2:    engine for the j=2 product when fin != 'V' ('A'/'V'), else '-'


def _spread(counts):
    """Build a 24-chain code list interleaving the given {code: count}."""
    codes = []
    for code, cnt in counts.items():
        codes.extend([code] * cnt)
    assert len(codes) == N_GROUPS * N_HALVES * 3, len(codes)
    # interleave so consecutive chains mix engine types
    out = []
    step = 7  # coprime with 24
    idx = 0
    taken = [False] * len(codes)
    for _ in range(len(codes)):
        while taken[idx]:
            idx = (idx + 1) % len(codes)
        out.append(codes[idx])
        taken[idx] = True
        idx = (idx + step) % len(codes)
    return out

# Config A (no GPSIMD): balance ACT (2 products on 22 chains) vs DVE.
CONFIG_A = _spread({"AAVV-": 22, "VVVV-": 2})
# Config B (with GPSIMD adds/finals).
CONFIG_B = _spread({"AVVV-": 8, "AAVV-": 6, "AAPV-": 6, "AVVPV": 4})

CHAIN_CODES = CONFIG_A


def build_bass(reps: int = 1, ops_mode: str = "full", chain_codes=None):
    if chain_codes is None:
        chain_codes = CHAIN_CODES
    nc = bacc.Bacc(
        "TRN2",
        debug=False,
        enable_asserts=False,
        target_bir_lowering=False,
        num_devices=N_CORES,
    )
    f32 = mybir.dt.float32
    bf16 = mybir.dt.bfloat16
    mult = mybir.AluOpType.mult
    add = mybir.AluOpType.add
    copy = mybir.ActivationFunctionType.Copy

    # xs[p = a*SEGS + s, col = g*9 + i*3 + j] = x[b, g*8 + a, i, j]
    xs = nc.dram_tensor("xs", [P, N_GROUPS * 9], f32, kind="ExternalInput").ap()
    # yt[j, m] = y[b, m, j]  (bf16 planes)
    yt = nc.dram_tensor("yt", [3, M], bf16, kind="ExternalInput").ap()
    out = nc.dram_tensor("out", [N_PER_CORE, M, 3], f32, kind="ExternalOutput").ap()

    with TileContext(nc) as tc:
        with (
            tc.tile_pool(name="const", bufs=1) as cpool,
            tc.tile_pool(name="tmpp", bufs=2) as tpool,
            tc.tile_pool(name="outp", bufs=2) as opool,
        ):
            xsb = cpool.tile([P, N_GROUPS * 9], f32)
            nc.sync.dma_start(out=xsb[:], in_=xs)

            # y resident in SBUF as bf16 planes: partition (a, s) holds
            # [j, t] = yt[j, s*ROWS + t]; replicated over the 8 a-groups.
            # Loaded in halves so unit (g=0, h=0) can start early.
            y_tile = cpool.tile([P, 3 * ROWS], bf16)
            yv = y_tile.rearrange("p (j t) -> p j t", j=3)
            y_src = yt.rearrange("j (s t) -> j s t", s=SEGS)  # [3, 16, 6250]
            for h in range(N_HALVES):
                t0, t1 = BOUNDS[h], BOUNDS[h + 1]
                for j in range(3):
                    nc.sync.dma_start(
                        out=yv[:, j, t0:t1],
                        in_=y_src[j, :, t0:t1].unsqueeze(0)
                        .to_broadcast((N_SUB, SEGS, t1 - t0)),
                    )

            units = [(g, h) for g in range(N_GROUPS) for h in range(N_HALVES)]
            units = units * reps

            def emit_products(u, g, h):
                t0, t1 = BOUNDS[h], BOUNDS[h + 1]
                nt = t1 - t0
                ys = [yv[:, j, t0:t1] for j in range(3)]
                prods = []
                for i in range(3):
                    code = chain_codes[((g * N_HALVES + h) * 3 + i) % 24]
                    c = g * 9 + i * 3
                    a = tpool.tile([P, nt], bf16, name=f"a{i}", tag=f"a{i}")
                    b = tpool.tile([P, nt], bf16, name=f"b{i}", tag=f"b{i}")
                    for slot, (eng, j, dst) in enumerate(
                        [(code[0], 0, a), (code[1], 1, b)]
                    ):
                        if eng == "A":
                            nc.scalar.activation(
                                out=dst[:], in_=ys[j], func=copy,
                                scale=xsb[:, c + j:c + j + 1])
                        else:
                            nc.vector.tensor_scalar(
                                out=dst[:], in0=ys[j],
                                scalar1=xsb[:, c + j:c + j + 1], scalar2=None,
                                op0=mult)
                    prods.append((code, c, a, b))
                return prods

            def emit_accums(u, g, h, prods):
                t0, t1 = BOUNDS[h], BOUNDS[h + 1]
                nt = t1 - t0
                ot = opool.tile([P, nt * 3], f32, name="ot", tag="ot")
                ov = ot.rearrange("p (t i) -> p t i", i=3)
                ys = [yv[:, j, t0:t1] for j in range(3)]
                for i in range(3):
                    code, c, a, b = prods[i]
                    if code[2] == "V":
                        nc.vector.tensor_tensor(out=a[:], in0=a[:], in1=b[:],
                                                op=add)
                    else:
                        nc.gpsimd.tensor_tensor(out=a[:], in0=a[:], in1=b[:],
                                                op=add)
                for i in range(3):
                    code, c, a, b = prods[i]
                    if code[3] == "V":
                        # out_i = (y2 * x2) + (j0 + j1): fuses the third
                        # product into the strided fp32 interleave write.
                        nc.vector.scalar_tensor_tensor(
                            out=ov[:, :, i], in0=ys[2],
                            scalar=xsb[:, c + 2:c + 3], in1=a[:],
                            op0=mult, op1=add)
                        continue
                    b2 = tpool.tile([P, nt], bf16, name=f"c{i}", tag=f"c{i}",
                                    bufs=1)
                    if code[4] == "A":
                        nc.scalar.activation(
                            out=b2[:], in_=ys[2], func=copy,
                            scale=xsb[:, c + 2:c + 3])
                    else:
                        nc.vector.tensor_scalar(
                            out=b2[:], in0=ys[2],
                            scalar1=xsb[:, c + 2:c + 3], scalar2=None,
                            op0=mult)
                    if code[3] == "P":
                        # GPSIMD does add + interleave in one op.
                        nc.gpsimd.tensor_tensor(
                            out=ov[:, :, i], in0=a[:], in1=b2[:], op=add)
                    else:
                        # bf16 2x add, then 1x strided upcast copy off-DVE.
                        nc.vector.tensor_tensor(out=a[:], in0=a[:], in1=b2[:],
                                                op=add)
                        if code[3] == "A":
                            nc.scalar.activation(out=ov[:, :, i], in_=a[:],
                                                 func=copy)
                        else:
                            nc.gpsimd.tensor_copy(out=ov[:, :, i], in_=a[:])
                dst = out[g * N_SUB:(g + 1) * N_SUB, :, :].rearrange(
                    "a (s t) i -> (a s) t i", s=SEGS
                )[:, t0:t1, :]
                nc.sync.dma_start(out=dst, in_=ot[:])

            def emit_unit_none(u, g, h):
                t0, t1 = BOUNDS[h], BOUNDS[h + 1]
                nt = t1 - t0
                ot = opool.tile([P, nt * 3], f32, name="ot", tag="ot")
                nc.vector.memset(ot[:], 0.0)
                dst = out[g * N_SUB:(g + 1) * N_SUB, :, :].rearrange(
                    "a (s t) i -> (a s) t i", s=SEGS
                )[:, t0:t1, :]
                nc.sync.dma_start(out=dst, in_=ot[:])

            if ops_mode == "full":
                # Software-pipelined emission: unit u+1's products are
                # issued before unit u's accumulates so the ACT stream
                # runs a full unit ahead of the DVE stream.
                prev = None
                for u, (g, h) in enumerate(units):
                    prods = emit_products(u, g, h)
                    if prev is not None:
                        emit_accums(*prev)
                    prev = (u, g, h, prods)
                if prev is not None:
                    emit_accums(*prev)
            elif ops_mode == "none":
                for u, (g, h) in enumerate(units):
                    emit_unit_none(u, g, h)
            else:
                raise ValueError(ops_mode)
    nc.compile()
    return nc


def _make_in_maps(x, y):
    x_flat = x.reshape(256, 3, 3)
    in_maps = []
    for c in range(N_CORES):
        b = c // 2
        xl = x_flat[c * N_PER_CORE:(c + 1) * N_PER_CORE]  # [32, 3, 3]
        per_a = xl.reshape(N_GROUPS, N_SUB, 9).transpose(1, 0, 2)  # [a, g, 9]
        per_a = per_a.reshape(N_SUB, N_GROUPS * 9)
        xsb_np = np.ascontiguousarray(np.repeat(per_a, SEGS, axis=0))  # [128, 36]
        yt_np = np.ascontiguousarray(y[b].T).astype(ml_dtypes.bfloat16)
        in_maps.append({"xs": xsb_np, "yt": yt_np})
    return in_maps


def kernel(x: np.ndarray, y: np.ndarray) -> np.ndarray:
    global LAST, _CACHED_NC
    x = np.ascontiguousarray(x, dtype=np.float32)
    y = np.ascontiguousarray(y, dtype=np.float32)
    assert x.shape == (4, 64, 3, 3) and y.shape == (4, 100000, 3)

    if _CACHED_NC is None:
        _CACHED_NC = build_bass()
    nc = _CACHED_NC

    in_maps = _make_in_maps(x, y)
    res = run_bass_kernel_spmd(
        nc, in_maps, core_ids=list(range(N_CORES)), trace=TRACE,
    )
    LAST = res
    out = np.concatenate([r["out"] for r in res.results], axis=0)
    return out.reshape(4, 64, 100000, 3)


def _prepare_exec(nc, in_maps, block=True):
    """Build a jitted 8-core executor for `nc` with device-resident inputs."""
    import jax
    import concourse.mybir as mybir_
    from jax.experimental.shard_map import shard_map
    from jax.sharding import Mesh, NamedSharding, PartitionSpec
    from concourse.bass2jax import (
        _bass_exec_p, install_neuronx_cc_hook, partition_id_tensor,
    )

    install_neuronx_cc_hook()
    partition_name = nc.partition_id_tensor.name if nc.partition_id_tensor else None
    in_names, out_names, out_avals, zero_outs = [], [], [], []
    for alloc in nc.m.functions[0].allocations:
        if not isinstance(alloc, mybir_.MemoryLocationSet):
            continue
        name = alloc.memorylocations[0].name
        if alloc.kind == "ExternalInput":
            if name != partition_name:
                in_names.append(name)
        elif alloc.kind == "ExternalOutput":
            shape = tuple(alloc.tensor_shape)
            dtype = mybir_.dt.np(alloc.dtype)
            out_names.append(name)
            out_avals.append(jax.core.ShapedArray(shape, dtype))
            zero_outs.append(np.zeros(shape, dtype))
    n_params = len(in_names)
    n_outs = len(out_names)
    all_names = in_names + out_names + ([partition_name] if partition_name else [])

    def _body(*args):
        operands = list(args)
        if partition_name is not None:
            operands.append(partition_id_tensor())
        outs = _bass_exec_p.bind(
            *operands,
            out_avals=tuple(out_avals),
            in_names=tuple(all_names),
            out_names=tuple(out_names),
            lowering_input_output_aliases=(),
            sim_require_finite=True,
            sim_require_nnan=True,
            nc=nc,
        )
        return tuple(outs)

    devices = jax.devices()[:N_CORES]
    mesh = Mesh(np.asarray(devices), ("core",))
    spec = PartitionSpec("core")
    sharded = jax.jit(
        shard_map(
            _body, mesh=mesh, in_specs=(spec,) * (n_params + n_outs),
            out_specs=(spec,) * n_outs, check_rep=False,
        ),
        donate_argnums=tuple(range(n_params, n_params + n_outs)),
        keep_unused=True,
    )
    sh = NamedSharding(mesh, spec)
    ins_dev = [
        jax.device_put(
            np.concatenate([np.asarray(m[name]) for m in in_maps], axis=0), sh
        )
        for name in in_names
    ]
    zeros = [
        jax.device_put(
            np.zeros((N_CORES * z.shape[0], *z.shape[1:]), z.dtype), sh
        )
        for z in zero_outs
    ]

    def run_once(outs):
        res = sharded(*ins_dev, *outs)
        if block:
            jax.block_until_ready(res)
        return list(res)

    return run_once, zeros


def bench(x, y, reps_pair=(9, 65), samples=24, ops_mode="full", chain_codes=None):
    """Measure steady-state per-workload HW time by differencing kernels
    that run the workload `reps_pair[0]` vs `reps_pair[1]` times.

    The host<->device tunnel sync costs tens of ms with heavy jitter,
    dwarfing the ~1-8 ms device time of a single execution, so per-call
    wall-clock differencing is unusable. Instead we enqueue chains of
    executions WITHOUT intermediate blocking: each call consumes the
    previous call's donated output buffers, so the device must run them
    serially while the host runs ahead; one sync at the end. Differencing
    two chain lengths cancels the sync + dispatch overhead, and the
    workload-reps differencing on top cancels any per-execution device
    overhead: t = [T(n2,r2)-T(n1,r2)] - [T(n2,r1)-T(n1,r1)] scaled."""
    import time
    x = np.ascontiguousarray(x, dtype=np.float32)
    y = np.ascontiguousarray(y, dtype=np.float32)
    in_maps = _make_in_maps(x, y)
    rounds = 6
    slope = {}
    for reps in reps_pair:
        # chain lengths: keep the timed span ~60+ ms so enqueue jitter
        # stays small relative to the device-side signal
        n1, n2 = 4, (48 if reps <= 16 else 24)
        nc = build_bass(reps=reps, ops_mode=ops_mode, chain_codes=chain_codes)
        run, zeros = _prepare_exec(nc, in_maps, block=False)
        import jax
        outs = run(zeros)
        jax.block_until_ready(outs)  # compile + warm
        slopes = []
        for _ in range(rounds):
            ts = {}
            for n in (n1, n2):
                jax.block_until_ready(outs)
                t0 = time.perf_counter()
                for _ in range(n):
                    outs = run(outs)
                jax.block_until_ready(outs)
                ts[n] = time.perf_counter() - t0
            slopes.append((ts[n2] - ts[n1]) / (n2 - n1))
        slopes.sort()
        med = slopes[len(slopes) // 2]
        slope[reps] = min(slopes)
        print(f"reps={reps}: per-exec slope min {slope[reps]*1e3:.3f} ms  "
              f"med {med*1e3:.3f}  all {[f'{s*1e3:.2f}' for s in slopes]}")
    r1, r2 = reps_pair
    per_iter = (slope[r2] - slope[r1]) / (r2 - r1) * 1e9
    print(f"per-iter (chained-exec slope diff): {per_iter:.0f} ns")
    return per_iter


# revision 19
# speedup vs baseline: 1.0044x; 1.0044x over previous
"""Trainium2 Bass kernel for nn_BatchMatMulModule.

Computes out = einsum("bnij,bmj->bnmi", x, y) with
  x: [4, 64, 3, 3] f32, y: [4, 100000, 3] f32 -> out: [4, 64, 100000, 3] f32.

The output (307 MB) dwarfs the inputs (4.8 MB); per-core store floor is
~107 us (38.4 MB at ~358 GB/s HBM per NeuronCore). The v1 kernel was
DVE-bound at ~165 us because both accumulate passes were fp32
scalar_tensor_tensor ops (1x mode, ~1.04 ns/elem each). This version
restructures the compute around DVE perf modes:

- y is shipped from the host as bf16 *planes* (yt = y[b].T, [3, 100000])
  so every compute read is contiguous; x scalars stay fp32 (scalar
  operands are exempt from the 2x-mode dtype rule).
- Per output element: two bf16 products (DVE tensor_scalar runs 4x: 0.26
  ns/elem; or ACT activation: 1x @1.2 GHz, 0.83 ns/elem), one bf16
  tensor_tensor add (DVE 2x: 0.52 ns/elem), and one final
  scalar_tensor_tensor that fuses the third product with the accumulate
  and the strided fp32 interleave write (1x, 1.04 ns/elem - the
  interleave into the [.., m, 3] output layout forces 1x regardless, so
  only ONE such pass remains vs v1's two).
- Chains are assigned per-engine via CHAIN_CODES to balance ACT vs DVE
  busy time. GPSIMD routing (add='P', fin='P'/'Q') was measured on HW and
  REGRESSES badly (127us -> 180+us for 12 GPS adds): the Q7 cores share
  their SBUF port with the DVE, so Pool-engine tensor ops stall the
  saturated DVE. Keep GPSIMD out of the hot loop.

The balanced ACT/DVE floor of this decomposition is ~126 us busy per
engine (store floor ~107 us at ~358 GB/s); measured ~127 us steady-state
per workload on quiet hardware.

Sharding: core c handles b = c // 2, n in [32 * (c % 2), ...) as in v1.
Per core, partition p = (a in 0..8, s in 0..16); group g covers n =
g*8 + a; segment s covers y rows [s*6250, (s+1)*6250); each unit
(g, h in 0..2) computes rows bounds[h]..bounds[h+1] (3124/3126 split so
bf16 packed modes keep even element counts and 4B alignment).
"""

import numpy as np
import ml_dtypes

import concourse.bacc as bacc
import concourse.mybir as mybir
from concourse.bass_utils import run_bass_kernel_spmd
from concourse.tile import TileContext

N_CORES = 8
P = 128
N_PER_CORE = 32
N_SUB = 8            # n values per group (partition-major)
SEGS = 16            # m segments per partition group
N_GROUPS = N_PER_CORE // N_SUB   # 4
M = 100000
ROWS = M // SEGS     # 6250 rows per segment (even)
BOUNDS = (0, 3124, ROWS)  # unit halves; both chunks even-sized
N_HALVES = len(BOUNDS) - 1

TRACE = False
LAST = None

_CACHED_NC = None

# Per-chain engine assignment. Chain index = (g * N_HALVES + h) * 3 + i.
# Code = 5 chars (p0, p1, add, fin, p2):
#   p0/p1: engine for the j=0 / j=1 products: 'A' (ACT) or 'V' (DVE ts)
#   add:   engine for the j0+j1 add: 'V' (DVE tt) or 'P' (GPSIMD tt)
#   fin:   final op producing ov[:, :, i] (fp32, stride-3 interleave):
#          'V' = DVE stt (fuses j2 product into the 1x strided write)
#          'P' = GPSIMD tt a+b2 (needs explicit j2 product, see p2)
#          'A' = DVE tt a+b2 -> v (bf16 2x), then ACT strided upcast copy
#          'Q' = like 'A' but the strided copy runs on GPSIMD
#   p2:    engine for the j=2 product when fin != 'V' ('A'/'V'), else '-'


def _spread(counts):
    """Build a 24-chain code list interleaving the given {code: count}."""
    codes = []
    for code, cnt in counts.items():
        codes.extend([code] * cnt)
    assert len(codes) == N_GROUPS * N_HALVES * 3, len(codes)
    # interleave so consecutive chains mix engine types
    out = []
    step = 7  # coprime with 24
    idx = 0
    taken = [False] * len(codes)
    for _ in range(len(codes)):
        while taken[idx]:
            idx = (idx + 1) % len(codes)
        out.append(codes[idx])
        taken[idx] = True
        idx = (idx + step) % len(codes)
    return out

# Config A (no GPSIMD): balance ACT (2 products on 22 chains) vs DVE.
CONFIG_A = _spread({"AAVV-": 22, "VVVV-": 2})
# Config B (with GPSIMD adds/finals).
CONFIG_B = _spread({"AVVV-": 8, "AAVV-": 6, "AAPV-": 6, "AVVPV": 4})

CHAIN_CODES = CONFIG_A


def build_bass(reps: int = 1, ops_mode: str = "full", chain_codes=None):
    if chain_codes is None:
        chain_codes = CHAIN_CODES
    nc = bacc.Bacc(
        "TRN2",
        debug=False,
        enable_asserts=False,
        target_bir_lowering=False,
        num_devices=N_CORES,
    )
    f32 = mybir.dt.float32
    bf16 = mybir.dt.bfloat16
    mult = mybir.AluOpType.mult
    add = mybir.AluOpType.add
    copy = mybir.ActivationFunctionType.Copy

    # xs[p = a*SEGS + s, col = g*9 + i*3 + j] = x[b, g*8 + a, i, j]
    xs = nc.dram_tensor("xs", [P, N_GROUPS * 9], f32, kind="ExternalInput").ap()
    # yt[j, m] = y[b, m, j]  (bf16 planes)
    yt = nc.dram_tensor("yt", [3, M], bf16, kind="ExternalInput").ap()
    out = nc.dram_tensor("out", [N_PER_CORE, M, 3], f32, kind="ExternalOutput").ap()

    with TileContext(nc) as tc:
        with (
            tc.tile_pool(name="const", bufs=1) as cpool,
            tc.tile_pool(name="tmpp", bufs=2) as tpool,
            tc.tile_pool(name="outp", bufs=2) as opool,
        ):
            xsb = cpool.tile([P, N_GROUPS * 9], f32)
            nc.sync.dma_start(out=xsb[:], in_=xs)

            # y resident in SBUF as bf16 planes: partition (a, s) holds
            # [j, t] = yt[j, s*ROWS + t]; replicated over the 8 a-groups.
            # Loaded in halves so unit (g=0, h=0) can start early.
            y_tile = cpool.tile([P, 3 * ROWS], bf16)
            yv = y_tile.rearrange("p (j t) -> p j t", j=3)
            y_src = yt.rearrange("j (s t) -> j s t", s=SEGS)  # [3, 16, 6250]
            for h in range(N_HALVES):
                t0, t1 = BOUNDS[h], BOUNDS[h + 1]
                for j in range(3):
                    nc.sync.dma_start(
                        out=yv[:, j, t0:t1],
                        in_=y_src[j, :, t0:t1].unsqueeze(0)
                        .to_broadcast((N_SUB, SEGS, t1 - t0)),
                    )

            units = [(g, h) for g in range(N_GROUPS) for h in range(N_HALVES)]
            units = units * reps

            def emit_products(u, g, h):
                t0, t1 = BOUNDS[h], BOUNDS[h + 1]
                nt = t1 - t0
                ys = [yv[:, j, t0:t1] for j in range(3)]
                # One wide A/B tile per unit: the 3 chains' products land in
                # adjacent nt-slices so the j0+j1 adds can run as a single
                # [P, 3*nt] tensor_tensor (fewer DVE instructions - the HW
                # pays ~200ns fixed cost per DVE op beyond the cost model).
                A = tpool.tile([P, nt * 3], bf16, name="A", tag="A")
                B = tpool.tile([P, nt * 3], bf16, name="B", tag="B")
                prods = []
                for i in range(3):
                    code = chain_codes[((g * N_HALVES + h) * 3 + i) % 24]
                    c = g * 9 + i * 3
                    a = A[:, i * nt:(i + 1) * nt]
                    b = B[:, i * nt:(i + 1) * nt]
                    for slot, (eng, j, dst) in enumerate(
                        [(code[0], 0, a), (code[1], 1, b)]
                    ):
                        if eng == "A":
                            nc.scalar.activation(
                                out=dst, in_=ys[j], func=copy,
                                scale=xsb[:, c + j:c + j + 1])
                        else:
                            nc.vector.tensor_scalar(
                                out=dst, in0=ys[j],
                                scalar1=xsb[:, c + j:c + j + 1], scalar2=None,
                                op0=mult)
                    prods.append((code, c, a, b))
                return prods, A, B

            def emit_accums(u, g, h, prods, A, B):
                t0, t1 = BOUNDS[h], BOUNDS[h + 1]
                nt = t1 - t0
                ot = opool.tile([P, nt * 3], f32, name="ot", tag="ot")
                ov = ot.rearrange("p (t i) -> p t i", i=3)
                ys = [yv[:, j, t0:t1] for j in range(3)]
                if all(pr[0][2] == "V" for pr in prods):
                    # single wide bf16 2x add covering all three chains
                    nc.vector.tensor_tensor(out=A[:], in0=A[:], in1=B[:],
                                            op=add)
                else:
                    for i in range(3):
                        code, c, a, b = prods[i]
                        if code[2] == "V":
                            nc.vector.tensor_tensor(out=a, in0=a, in1=b,
                                                    op=add)
                        else:
                            nc.gpsimd.tensor_tensor(out=a, in0=a, in1=b,
                                                    op=add)
                for i in range(3):
                    code, c, a, b = prods[i]
                    if code[3] == "V":
                        # out_i = (y2 * x2) + (j0 + j1): fuses the third
                        # product into the strided fp32 interleave write.
                        nc.vector.scalar_tensor_tensor(
                            out=ov[:, :, i], in0=ys[2],
                            scalar=xsb[:, c + 2:c + 3], in1=a,
                            op0=mult, op1=add)
                        continue
                    b2 = tpool.tile([P, nt], bf16, name=f"c{i}", tag=f"c{i}",
                                    bufs=1)
                    if code[4] == "A":
                        nc.scalar.activation(
                            out=b2[:], in_=ys[2], func=copy,
                            scale=xsb[:, c + 2:c + 3])
                    else:
                        nc.vector.tensor_scalar(
                            out=b2[:], in0=ys[2],
                            scalar1=xsb[:, c + 2:c + 3], scalar2=None,
                            op0=mult)
                    if code[3] == "P":
                        # GPSIMD does add + interleave in one op.
                        nc.gpsimd.tensor_tensor(
                            out=ov[:, :, i], in0=a, in1=b2[:], op=add)
                    else:
                        # bf16 2x add, then 1x strided upcast copy off-DVE.
                        nc.vector.tensor_tensor(out=a, in0=a, in1=b2[:],
                                                op=add)
                        if code[3] == "A":
                            nc.scalar.activation(out=ov[:, :, i], in_=a,
                                                 func=copy)
                        else:
                            nc.gpsimd.tensor_copy(out=ov[:, :, i], in_=a)
                dst = out[g * N_SUB:(g + 1) * N_SUB, :, :].rearrange(
                    "a (s t) i -> (a s) t i", s=SEGS
                )[:, t0:t1, :]
                nc.sync.dma_start(out=dst, in_=ot[:])

            def emit_unit_none(u, g, h):
                t0, t1 = BOUNDS[h], BOUNDS[h + 1]
                nt = t1 - t0
                ot = opool.tile([P, nt * 3], f32, name="ot", tag="ot")
                nc.vector.memset(ot[:], 0.0)
                dst = out[g * N_SUB:(g + 1) * N_SUB, :, :].rearrange(
                    "a (s t) i -> (a s) t i", s=SEGS
                )[:, t0:t1, :]
                nc.sync.dma_start(out=dst, in_=ot[:])

            if ops_mode == "full":
                # Per-unit emission; the tile scheduler already overlaps
                # unit u+1's ACT products with unit u's DVE accumulates
                # (explicit software pipelining was sim- and HW-neutral).
                for u, (g, h) in enumerate(units):
                    prods, A, B = emit_products(u, g, h)
                    emit_accums(u, g, h, prods, A, B)
            elif ops_mode == "none":
                for u, (g, h) in enumerate(units):
                    emit_unit_none(u, g, h)
            else:
                raise ValueError(ops_mode)
    nc.compile()
    return nc


def _make_in_maps(x, y):
    x_flat = x.reshape(256, 3, 3)
    in_maps = []
    for c in range(N_CORES):
        b = c // 2
        xl = x_flat[c * N_PER_CORE:(c + 1) * N_PER_CORE]  # [32, 3, 3]
        per_a = xl.reshape(N_GROUPS, N_SUB, 9).transpose(1, 0, 2)  # [a, g, 9]
        per_a = per_a.reshape(N_SUB, N_GROUPS * 9)
        xsb_np = np.ascontiguousarray(np.repeat(per_a, SEGS, axis=0))  # [128, 36]
        yt_np = np.ascontiguousarray(y[b].T).astype(ml_dtypes.bfloat16)
        in_maps.append({"xs": xsb_np, "yt": yt_np})
    return in_maps


def kernel(x: np.ndarray, y: np.ndarray) -> np.ndarray:
    global LAST, _CACHED_NC
    x = np.ascontiguousarray(x, dtype=np.float32)
    y = np.ascontiguousarray(y, dtype=np.float32)
    assert x.shape == (4, 64, 3, 3) and y.shape == (4, 100000, 3)

    if _CACHED_NC is None:
        _CACHED_NC = build_bass()
    nc = _CACHED_NC

    in_maps = _make_in_maps(x, y)
    res = run_bass_kernel_spmd(
        nc, in_maps, core_ids=list(range(N_CORES)), trace=TRACE,
    )
    LAST = res
    out = np.concatenate([r["out"] for r in res.results], axis=0)
    return out.reshape(4, 64, 100000, 3)


def _prepare_exec(nc, in_maps, block=True):
    """Build a jitted 8-core executor for `nc` with device-resident inputs."""
    import jax
    import concourse.mybir as mybir_
    from jax.experimental.shard_map import shard_map
    from jax.sharding import Mesh, NamedSharding, PartitionSpec
    from concourse.bass2jax import (
        _bass_exec_p, install_neuronx_cc_hook, partition_id_tensor,
    )

    install_neuronx_cc_hook()
    partition_name = nc.partition_id_tensor.name if nc.partition_id_tensor else None
    in_names, out_names, out_avals, zero_outs = [], [], [], []
    for alloc in nc.m.functions[0].allocations:
        if not isinstance(alloc, mybir_.MemoryLocationSet):
            continue
        name = alloc.memorylocations[0].name
        if alloc.kind == "ExternalInput":
            if name != partition_name:
                in_names.append(name)
        elif alloc.kind == "ExternalOutput":
            shape = tuple(alloc.tensor_shape)
            dtype = mybir_.dt.np(alloc.dtype)
            out_names.append(name)
            out_avals.append(jax.core.ShapedArray(shape, dtype))
            zero_outs.append(np.zeros(shape, dtype))
    n_params = len(in_names)
    n_outs = len(out_names)
    all_names = in_names + out_names + ([partition_name] if partition_name else [])

    def _body(*args):
        operands = list(args)
        if partition_name is not None:
            operands.append(partition_id_tensor())
        outs = _bass_exec_p.bind(
            *operands,
            out_avals=tuple(out_avals),
            in_names=tuple(all_names),
            out_names=tuple(out_names),
            lowering_input_output_aliases=(),
            sim_require_finite=True,
            sim_require_nnan=True,
            nc=nc,
        )
        return tuple(outs)

    devices = jax.devices()[:N_CORES]
    mesh = Mesh(np.asarray(devices), ("core",))
    spec = PartitionSpec("core")
    sharded = jax.jit(
        shard_map(
            _body, mesh=mesh, in_specs=(spec,) * (n_params + n_outs),
            out_specs=(spec,) * n_outs, check_rep=False,
        ),
        donate_argnums=tuple(range(n_params, n_params + n_outs)),
        keep_unused=True,
    )
    sh = NamedSharding(mesh, spec)
    ins_dev = [
        jax.device_put(
            np.concatenate([np.asarray(m[name]) for m in in_maps], axis=0), sh
        )
        for name in in_names
    ]
    zeros = [
        jax.device_put(
            np.zeros((N_CORES * z.shape[0], *z.shape[1:]), z.dtype), sh
        )
        for z in zero_outs
    ]

    def run_once(outs):
        res = sharded(*ins_dev, *outs)
        if block:
            jax.block_until_ready(res)
        return list(res)

    return run_once, zeros


def bench(x, y, reps_pair=(9, 65), samples=24, ops_mode="full", chain_codes=None):
    """Measure steady-state per-workload HW time by differencing kernels
    that run the workload `reps_pair[0]` vs `reps_pair[1]` times.

    The host<->device tunnel sync costs tens of ms with heavy jitter,
    dwarfing the ~1-8 ms device time of a single execution, so per-call
    wall-clock differencing is unusable. Instead we enqueue chains of
    executions WITHOUT intermediate blocking: each call consumes the
    previous call's donated output buffers, so the device must run them
    serially while the host runs ahead; one sync at the end. Differencing
    two chain lengths cancels the sync + dispatch overhead, and the
    workload-reps differencing on top cancels any per-execution device
    overhead: t = [T(n2,r2)-T(n1,r2)] - [T(n2,r1)-T(n1,r1)] scaled."""
    import time
    x = np.ascontiguousarray(x, dtype=np.float32)
    y = np.ascontiguousarray(y, dtype=np.float32)
    in_maps = _make_in_maps(x, y)
    rounds = 6
    slope = {}
    for reps in reps_pair:
        # chain lengths: keep the timed span ~60+ ms so enqueue jitter
        # stays small relative to the device-side signal
        n1, n2 = 4, (48 if reps <= 16 else 24)
        nc = build_bass(reps=reps, ops_mode=ops_mode, chain_codes=chain_codes)
        run, zeros = _prepare_exec(nc, in_maps, block=False)
        import jax
        outs = run(zeros)
        jax.block_until_ready(outs)  # compile + warm
        slopes = []
        for _ in range(rounds):
            ts = {}
            for n in (n1, n2):
                jax.block_until_ready(outs)
                t0 = time.perf_counter()
                for _ in range(n):
                    outs = run(outs)
                jax.block_until_ready(outs)
                ts[n] = time.perf_counter() - t0
            slopes.append((ts[n2] - ts[n1]) / (n2 - n1))
        slopes.sort()
        med = slopes[len(slopes) // 2]
        slope[reps] = min(slopes)
        print(f"reps={reps}: per-exec slope min {slope[reps]*1e3:.3f} ms  "
              f"med {med*1e3:.3f}  all {[f'{s*1e3:.2f}' for s in slopes]}")
    r1, r2 = reps_pair
    per_iter = (slope[r2] - slope[r1]) / (r2 - r1) * 1e9
    print(f"per-iter (chained-exec slope diff): {per_iter:.0f} ns")
    return per_iter
